# revision 6
# baseline (speedup 1.0000x reference)
"""Trainium2 Bass kernel: AQT-style int8-quantized matmul, SPMD over 8 NeuronCores.

  out = (qlhs @ qrhs) * lhs_scale * rhs_scale
  lhs_scale = max(|lhs|,axis=1)/127, rhs_scale = max(|rhs|,axis=0)/127
  qx = round-half-even(x/scale) in [-127,127]

int8 values are exact in bf16 and all accumulations stay < 2^24, so a bf16
matmul with fp32 PSUM accumulation reproduces the int32 arithmetic exactly.

Sharding: M-parallel. Core c gets lhs rows [c*1024,(c+1)*1024), the full rhs,
and a per-core column slice rhs[:, c*512:(c+1)*512] as a separate input used to
compute column abs-max scales (sharded scan + 16KB AllGather). Output shards
concatenate along M.
"""
import sys

import numpy as np

for _p in ("/opt/trn_rl_repo", "/opt/pypackages"):
    if _p not in sys.path:
        sys.path.append(_p)

import concourse.bass as bass
import concourse.mybir as mybir
import concourse.tile as tile
from concourse import bacc

P = 128
MAGIC = 12582912.0          # 1.5 * 2^23: fp32 add/sub rounds to nearest-even integer
F32 = mybir.dt.float32
BF16 = mybir.dt.bfloat16
INV127 = float(np.float32(1.0) / np.float32(127.0))

N_CORES = 8
FULL_M = 8192
K_DIM = 4096
N_DIM = 4096


def build(n_cores=8, M=1024, K=4096, N=4096, NFREE=512,
          x_bufs=2, t_bufs=1, qm_bufs=2, qr_bufs=2, rst_bufs=4, tt_bufs=4,
          ps_bufs=4, o_bufs=3):
    """Build the SPMD Bass graph for one core (same graph runs on all cores).

    M: per-core lhs rows.  K: contraction.  N: full output columns.
    NFREE: matmul moving free dim (psum tile width).
    """
    KT = K // P                 # k-tiles
    MT = M // P                 # m-tiles
    NSCAN = N // n_cores        # columns scanned per core
    NCHUNKS = N // NFREE
    KJ = N // P                 # colmax vector viewed as [P, KJ]
    assert K % P == 0 and M % P == 0 and N % NFREE == 0 and N % n_cores == 0
    assert N % P == 0 and NSCAN % P == 0

    nc = bacc.Bacc(None, target_bir_lowering=False, num_devices=n_cores)
    lhs = nc.declare_dram_parameter("lhs", [M, K], F32, isOutput=False)
    rhs = nc.declare_dram_parameter("rhs", [K, N], F32, isOutput=False)
    rhs_scan = nc.declare_dram_parameter("rhs_scan", [K, NSCAN], F32, isOutput=False)
    out = nc.declare_dram_parameter("out", [M, N], F32, isOutput=True)

    groups = [list(range(n_cores))]

    with tile.TileContext(nc, num_cores=n_cores) as tc:
        with tc.tile_pool(name="persist", bufs=1) as persist, \
             tc.tile_pool(name="dram", bufs=1, space="DRAM") as dram:
            qlhsT = []
            for mt in range(MT):
                ql = persist.tile([P, KT, P], BF16, tag=f"qlhsT{mt}", name=f"qlhsT{mt}")
                qlhsT.append(ql)
            r_bc = persist.tile([P, N], F32)     # 127/colmax, broadcast along partitions
            s_bc = persist.tile([P, N], F32)     # colmax/127, broadcast along partitions
            s_l = persist.tile([P, MT], F32)     # lhs scales per m-tile column

            # ---------------- Phase A: rhs column-scale scan (sharded) ----------------
            with tc.tile_pool(name="scanp", bufs=1) as scanp:
                acc = scanp.tile([P, NSCAN], F32, name="scan_acc")
                for kt in range(KT):
                    st = scanp.tile([P, NSCAN], F32, tag="scan_in", bufs=4,
                                    name=f"scan_in{kt}")
                    nc.sync.dma_start(st[:], rhs_scan[kt * P:(kt + 1) * P, :])
                    if kt == 0:
                        nc.scalar.activation(acc[:], st[:],
                                             mybir.ActivationFunctionType.Abs)
                    else:
                        nc.scalar.activation(st[:], st[:],
                                             mybir.ActivationFunctionType.Abs)
                        nc.vector.tensor_tensor(acc[:], acc[:], st[:],
                                                op=mybir.AluOpType.max)
                # partition-halving abs-max tree: colmax ends up in acc[0:1, :]
                h = P // 2
                lvl = 0
                while h >= 1:
                    ptree = scanp.tile([h, NSCAN], F32, tag="ptree", bufs=2,
                                       name=f"ptree{lvl}")
                    nc.sync.dma_start(ptree[:], acc[h:2 * h, :])
                    nc.vector.tensor_tensor(acc[0:h, :], acc[0:h, :], ptree[:],
                                            op=mybir.AluOpType.max)
                    h //= 2
                    lvl += 1
                cm_in = dram.tile([1, NSCAN], F32, name="cm_in")
                nc.sync.dma_start(cm_in[:], acc[0:1, :])
                cm_out = dram.tile([n_cores, NSCAN], F32, addr_space="Shared",
                                   name="cm_out")
                nc.gpsimd.collective_compute(
                    "AllGather", mybir.AluOpType.bypass, replica_groups=groups,
                    ins=[cm_in[:].opt()], outs=[cm_out[:].opt()])
                # colmax as [P, KJ] for a cheap reciprocal, then broadcast via DRAM
                cm_sb = scanp.tile([P, KJ], F32, name="cm_sb")
                nc.sync.dma_start(cm_sb[:], cm_out[:].rearrange("c n -> (c n)")
                                  .rearrange("(p j) -> p j", p=P))
                r_sm = scanp.tile([P, KJ], F32, name="r_sm")
                s_sm = scanp.tile([P, KJ], F32, name="s_sm")
                nc.vector.reciprocal(r_sm[:], cm_sb[:])
                nc.vector.tensor_scalar(r_sm[:], r_sm[:], 127.0, None,
                                        op0=mybir.AluOpType.mult)
                nc.vector.tensor_scalar(s_sm[:], cm_sb[:], INV127, None,
                                        op0=mybir.AluOpType.mult)
                r_dram = dram.tile([N], F32, name="r_dram")
                s_dram = dram.tile([N], F32, name="s_dram")
                nc.sync.dma_start(r_dram[:].rearrange("(p j) -> p j", p=P), r_sm[:])
                nc.sync.dma_start(s_dram[:].rearrange("(p j) -> p j", p=P), s_sm[:])
                nc.sync.dma_start(r_bc[:], r_dram[:].rearrange("(a n) -> a n", a=1)
                                  .partition_broadcast(P))
                nc.sync.dma_start(s_bc[:], s_dram[:].rearrange("(a n) -> a n", a=1)
                                  .partition_broadcast(P))

            # ---------------- Phase B: lhs quantize + transpose ----------------
            with tc.tile_pool(name="lhsp", bufs=1) as lhsp:
                for mt in range(MT):
                    xb = lhsp.tile([P, K], F32, tag="x", bufs=x_bufs, name=f"x{mt}")
                    nc.sync.dma_start(xb[:], lhs[mt * P:(mt + 1) * P, :])
                    rowmax = lhsp.tile([P, 1], F32, tag="rowmax", bufs=2,
                                       name=f"rowmax{mt}")
                    nc.vector.tensor_reduce(rowmax[:], xb[:], axis=mybir.AxisListType.X,
                                            op=mybir.AluOpType.max,
                                            apply_absolute_value=True)
                    nc.vector.tensor_scalar(s_l[:, mt:mt + 1], rowmax[:], INV127, None,
                                            op0=mybir.AluOpType.mult)
                    rl = lhsp.tile([P, 1], F32, tag="rl", bufs=2, name=f"rl{mt}")
                    nc.vector.reciprocal(rl[:], rowmax[:])
                    nc.vector.tensor_scalar(rl[:], rl[:], 127.0, None,
                                            op0=mybir.AluOpType.mult)
                    tb = lhsp.tile([P, K], F32, tag="t", bufs=t_bufs, name=f"t{mt}")
                    nc.scalar.activation(tb[:], xb[:], mybir.ActivationFunctionType.Copy,
                                         bias=MAGIC, scale=rl[:])
                    qm = lhsp.tile([P, K], BF16, tag="qm", bufs=qm_bufs, name=f"qm{mt}")
                    nc.vector.tensor_scalar(qm[:], tb[:], MAGIC, None,
                                            op0=mybir.AluOpType.subtract)
                    nc.sync.dma_start_transpose(out=qlhsT[mt][:], in_=qm[:])

            # ---------------- Phase C: stream rhs, quantize, matmul, dequant ----------------
            with tc.tile_pool(name="cp", bufs=1) as cp, \
                 tc.tile_pool(name="psump", bufs=1, space="PSUM") as psump:
                for nchu in range(NCHUNKS):
                    ncols = slice(nchu * NFREE, (nchu + 1) * NFREE)
                    qr = cp.tile([P, KT, NFREE], BF16, tag="qr", bufs=qr_bufs,
                                 name=f"qr{nchu}")
                    for kt in range(KT):
                        rst = cp.tile([P, NFREE], F32, tag="rst", bufs=rst_bufs,
                                      name=f"rst{nchu}_{kt}")
                        nc.sync.dma_start(rst[:], rhs[kt * P:(kt + 1) * P, ncols])
                        ttl = cp.tile([P, NFREE], F32, tag="tt", bufs=tt_bufs,
                                      name=f"tt{nchu}_{kt}")
                        nc.vector.tensor_tensor(ttl[:], rst[:], r_bc[:, ncols],
                                                op=mybir.AluOpType.mult)
                        nc.vector.tensor_scalar(qr[:, kt, :], ttl[:], MAGIC, MAGIC,
                                                op0=mybir.AluOpType.add,
                                                op1=mybir.AluOpType.subtract)
                    for mt in range(MT):
                        ps = psump.tile([P, NFREE], F32, tag="ps", bufs=ps_bufs,
                                        name=f"ps{nchu}_{mt}")
                        for kt in range(KT):
                            nc.tensor.matmul(ps[:], qlhsT[mt][:, kt, :], qr[:, kt, :],
                                             start=(kt == 0), stop=(kt == KT - 1))
                        o1 = cp.tile([P, NFREE], F32, tag="o1", bufs=o_bufs,
                                     name=f"o1_{nchu}_{mt}")
                        nc.vector.tensor_tensor(o1[:], ps[:], s_bc[:, ncols],
                                                op=mybir.AluOpType.mult)
                        o2 = cp.tile([P, NFREE], F32, tag="o2", bufs=o_bufs,
                                     name=f"o2_{nchu}_{mt}")
                        nc.scalar.activation(o2[:], o1[:],
                                             mybir.ActivationFunctionType.Copy,
                                             bias=0.0, scale=s_l[:, mt:mt + 1])
                        nc.sync.dma_start(out[mt * P:(mt + 1) * P, ncols], o2[:])
    nc.compile()
    return nc


def shard_inputs(lhs, rhs, n_cores=8):
    """Full inputs -> per-core in_maps."""
    M = lhs.shape[0] // n_cores
    NSCAN = rhs.shape[1] // n_cores
    return [
        {
            "lhs": np.ascontiguousarray(lhs[c * M:(c + 1) * M]),
            "rhs": rhs,
            "rhs_scan": np.ascontiguousarray(rhs[:, c * NSCAN:(c + 1) * NSCAN]),
        }
        for c in range(n_cores)
    ]


_NC_CACHE = {}


def _get_nc():
    key = "default"
    if key not in _NC_CACHE:
        _NC_CACHE[key] = build(n_cores=N_CORES, M=FULL_M // N_CORES, K=K_DIM, N=N_DIM)
    return _NC_CACHE[key]


def run_sharded(lhs, rhs, trace=False, **kwargs):
    """Run on hardware; returns (full_output, BassKernelResults)."""
    from concourse.bass_utils import run_bass_kernel_spmd
    nc = _get_nc()
    in_maps = shard_inputs(lhs, rhs, N_CORES)
    res = run_bass_kernel_spmd(nc, in_maps, core_ids=list(range(N_CORES)),
                               trace=trace, **kwargs)
    full = np.concatenate([res.results[c]["out"] for c in range(N_CORES)], axis=0)
    return full, res


def kernel(lhs, rhs):
    lhs = np.asarray(lhs, dtype=np.float32)
    rhs = np.asarray(rhs, dtype=np.float32)
    assert lhs.shape == (FULL_M, K_DIM) and rhs.shape == (K_DIM, N_DIM)
    full, _ = run_sharded(lhs, rhs, trace=False)
    return full


# revision 8
# speedup vs baseline: 1.0371x; 1.0371x over previous
"""Trainium2 Bass kernel: AQT-style int8-quantized matmul, SPMD over 8 NeuronCores.

  out = (qlhs @ qrhs) * lhs_scale * rhs_scale
  lhs_scale = max(|lhs|,axis=1)/127, rhs_scale = max(|rhs|,axis=0)/127
  qx = round-half-even(x/scale) in [-127,127]

int8 values are exact in bf16 and all accumulations stay < 2^24, so a bf16
matmul with fp32 PSUM accumulation reproduces the int32 arithmetic exactly.

Sharding: M-parallel. Core c gets lhs rows [c*1024,(c+1)*1024), the full rhs,
and a per-core column slice rhs[:, c*512:(c+1)*512] as a separate input used to
compute column abs-max scales (sharded scan + 16KB AllGather). Output shards
concatenate along M.
"""
import sys

import numpy as np

for _p in ("/opt/trn_rl_repo", "/opt/pypackages"):
    if _p not in sys.path:
        sys.path.append(_p)

import concourse.mybir as mybir
import concourse.tile as tile
from concourse import bacc

P = 128
MAGIC = 12582912.0          # 1.5 * 2^23: fp32 add/sub rounds to nearest-even integer
F32 = mybir.dt.float32
BF16 = mybir.dt.bfloat16
INV127 = float(np.float32(1.0) / np.float32(127.0))

N_CORES = 8
FULL_M = 8192
K_DIM = 4096
N_DIM = 4096


def build(n_cores=8, M=1024, K=4096, N=4096, NFREE=512, GKS=8, GK=4,
          st_bufs=2, x_bufs=2, qm_bufs=2, qr_bufs=2, rst_bufs=2,
          ps_bufs=4, o_bufs=3):
    """Build the SPMD Bass graph for one core (same graph runs on all cores).

    M: per-core lhs rows.  K: contraction.  N: full output columns.
    NFREE: matmul moving free dim.  GKS/GK: k-tile group sizes for the scan
    reduce and the rhs-quantize DVE ops.
    """
    KT = K // P                 # k-tiles
    MT = M // P                 # m-tiles
    NSCAN = N // n_cores        # columns scanned per core
    NCHUNKS = N // NFREE
    KJ = N // P                 # colmax vector viewed as [P, KJ]
    GKS = min(GKS, KT)
    GK = min(GK, KT)
    assert K % P == 0 and M % P == 0 and N % NFREE == 0 and N % n_cores == 0
    assert N % P == 0 and NSCAN % P == 0 and KT % GKS == 0 and KT % GK == 0

    nc = bacc.Bacc(None, target_bir_lowering=False, num_devices=n_cores)
    lhs = nc.declare_dram_parameter("lhs", [M, K], F32, isOutput=False)
    rhs = nc.declare_dram_parameter("rhs", [K, N], F32, isOutput=False)
    rhs_scan = nc.declare_dram_parameter("rhs_scan", [K, NSCAN], F32, isOutput=False)
    out = nc.declare_dram_parameter("out", [M, N], F32, isOutput=True)

    groups = [list(range(n_cores))]

    with tile.TileContext(nc, num_cores=n_cores, pool_alloc_mode="queue") as tc:
        with tc.tile_pool(name="persist", bufs=1) as persist, \
             tc.tile_pool(name="dram", bufs=1, space="DRAM") as dram:
            qlhsT = []
            for mt in range(MT):
                ql = persist.tile([P, KT, P], BF16, tag=f"qlhsT{mt}", name=f"qlhsT{mt}")
                qlhsT.append(ql)
            r_bc = persist.tile([P, N], F32)     # 127/colmax, bcast along partitions
            s_bc = persist.tile([P, N], F32)     # colmax/127, bcast along partitions
            s_l = persist.tile([P, MT], F32)     # lhs scales per m-tile column

            # -------- Phase A: rhs column-scale scan (sharded by column) --------
            with tc.tile_pool(name="scanp", bufs=1) as scanp:
                acc = scanp.tile([P, NSCAN], F32, name="scan_acc")
                for g in range(KT // GKS):
                    stb = scanp.tile([P, GKS, NSCAN], F32, tag="st", bufs=st_bufs,
                                     name=f"st{g}")
                    for i in range(GKS):
                        kt = g * GKS + i
                        nc.sync.dma_start(stb[:, i, :],
                                          rhs_scan[kt * P:(kt + 1) * P, :])
                    if g == 0:
                        nc.vector.tensor_reduce(
                            acc[:], stb[:].rearrange("p g f -> p f g"),
                            axis=mybir.AxisListType.X, op=mybir.AluOpType.max,
                            apply_absolute_value=True)
                    else:
                        gm = scanp.tile([P, NSCAN], F32, tag="gm", bufs=2,
                                        name=f"gm{g}")
                        nc.vector.tensor_reduce(
                            gm[:], stb[:].rearrange("p g f -> p f g"),
                            axis=mybir.AxisListType.X, op=mybir.AluOpType.max,
                            apply_absolute_value=True)
                        nc.vector.tensor_tensor(acc[:], acc[:], gm[:],
                                                op=mybir.AluOpType.max)
                # partition-halving max tree: colmax ends up in acc[0:1, :]
                h = P // 2
                lvl = 0
                while h >= 1:
                    ptree = scanp.tile([h, NSCAN], F32, tag="ptree", bufs=2,
                                       name=f"ptree{lvl}")
                    nc.sync.dma_start(ptree[:], acc[h:2 * h, :])
                    nc.vector.tensor_tensor(acc[0:h, :], acc[0:h, :], ptree[:],
                                            op=mybir.AluOpType.max)
                    h //= 2
                    lvl += 1
                cm_in = dram.tile([1, NSCAN], F32, name="cm_in")
                nc.sync.dma_start(cm_in[:], acc[0:1, :])
                cm_out = dram.tile([n_cores, NSCAN], F32, addr_space="Shared",
                                   name="cm_out")
                nc.gpsimd.collective_compute(
                    "AllGather", mybir.AluOpType.bypass, replica_groups=groups,
                    ins=[cm_in[:].opt()], outs=[cm_out[:].opt()])
                # colmax as [P, KJ] for a cheap reciprocal, then broadcast via DRAM
                cm_sb = scanp.tile([P, KJ], F32, name="cm_sb")
                nc.sync.dma_start(cm_sb[:], cm_out[:].rearrange("c n -> (c n)")
                                  .rearrange("(p j) -> p j", p=P))
                r_sm = scanp.tile([P, KJ], F32, name="r_sm")
                s_sm = scanp.tile([P, KJ], F32, name="s_sm")
                nc.vector.reciprocal(r_sm[:], cm_sb[:])
                nc.vector.tensor_scalar(r_sm[:], r_sm[:], 127.0, None,
                                        op0=mybir.AluOpType.mult)
                nc.vector.tensor_scalar(s_sm[:], cm_sb[:], INV127, None,
                                        op0=mybir.AluOpType.mult)
                r_dram = dram.tile([N], F32, name="r_dram")
                s_dram = dram.tile([N], F32, name="s_dram")
                nc.sync.dma_start(r_dram[:].rearrange("(p j) -> p j", p=P), r_sm[:])
                nc.sync.dma_start(s_dram[:].rearrange("(p j) -> p j", p=P), s_sm[:])
                nc.sync.dma_start(r_bc[:], r_dram[:].rearrange("(a n) -> a n", a=1)
                                  .partition_broadcast(P))
                nc.sync.dma_start(s_bc[:], s_dram[:].rearrange("(a n) -> a n", a=1)
                                  .partition_broadcast(P))

            # -------- Phase B: lhs quantize + transpose --------
            with tc.tile_pool(name="lhsp", bufs=1) as lhsp:
                for mt in range(MT):
                    xb = lhsp.tile([P, K], F32, tag="x", bufs=x_bufs, name=f"x{mt}")
                    nc.sync.dma_start(xb[:], lhs[mt * P:(mt + 1) * P, :])
                    rowmax = lhsp.tile([P, 1], F32, tag="rowmax", bufs=2,
                                       name=f"rowmax{mt}")
                    nc.vector.tensor_reduce(rowmax[:], xb[:], axis=mybir.AxisListType.X,
                                            op=mybir.AluOpType.max,
                                            apply_absolute_value=True)
                    nc.vector.tensor_scalar(s_l[:, mt:mt + 1], rowmax[:], INV127, None,
                                            op0=mybir.AluOpType.mult)
                    rl = lhsp.tile([P, 1], F32, tag="rl", bufs=2, name=f"rl{mt}")
                    nc.vector.reciprocal(rl[:], rowmax[:])
                    nc.vector.tensor_scalar(rl[:], rl[:], 127.0, None,
                                            op0=mybir.AluOpType.mult)
                    # in-place: x = x*rl + MAGIC (ACT), then qm = x - MAGIC (DVE)
                    nc.scalar.activation(xb[:], xb[:], mybir.ActivationFunctionType.Copy,
                                         bias=MAGIC, scale=rl[:])
                    qm = lhsp.tile([P, K], BF16, tag="qm", bufs=qm_bufs, name=f"qm{mt}")
                    nc.vector.tensor_scalar(qm[:], xb[:], MAGIC, None,
                                            op0=mybir.AluOpType.subtract)
                    nc.sync.dma_start_transpose(out=qlhsT[mt][:], in_=qm[:])

            # -------- Phase C: stream rhs, quantize, matmul, dequant --------
            with tc.tile_pool(name="cp", bufs=1) as cp, \
                 tc.tile_pool(name="psump", bufs=1, space="PSUM") as psump:
                for nchu in range(NCHUNKS):
                    ncols = slice(nchu * NFREE, (nchu + 1) * NFREE)
                    qr = cp.tile([P, KT, NFREE], BF16, tag="qr", bufs=qr_bufs,
                                 name=f"qr{nchu}")
                    r_slice = r_bc[:, ncols].rearrange("p f -> p () f") \
                                            .broadcast_to([P, GK, NFREE])
                    for g in range(KT // GK):
                        rst = cp.tile([P, GK, NFREE], F32, tag="rst", bufs=rst_bufs,
                                      name=f"rst{nchu}_{g}")
                        for i in range(GK):
                            kt = g * GK + i
                            nc.sync.dma_start(rst[:, i, :],
                                              rhs[kt * P:(kt + 1) * P, ncols])
                        nc.vector.tensor_tensor(rst[:], rst[:], r_slice,
                                                op=mybir.AluOpType.mult)
                        nc.vector.tensor_scalar(qr[:, g * GK:(g + 1) * GK, :], rst[:],
                                                MAGIC, MAGIC,
                                                op0=mybir.AluOpType.add,
                                                op1=mybir.AluOpType.subtract)
                    for mt in range(MT):
                        ps = psump.tile([P, NFREE], F32, tag="ps", bufs=ps_bufs,
                                        name=f"ps{nchu}_{mt}")
                        for kt in range(KT):
                            nc.tensor.matmul(ps[:], qlhsT[mt][:, kt, :], qr[:, kt, :],
                                             start=(kt == 0), stop=(kt == KT - 1))
                        o1 = cp.tile([P, NFREE], F32, tag="o1", bufs=o_bufs,
                                     name=f"o1_{nchu}_{mt}")
                        nc.scalar.activation(o1[:], ps[:],
                                             mybir.ActivationFunctionType.Copy,
                                             bias=0.0, scale=s_l[:, mt:mt + 1])
                        o2 = cp.tile([P, NFREE], F32, tag="o2", bufs=o_bufs,
                                     name=f"o2_{nchu}_{mt}")
                        nc.vector.tensor_tensor(o2[:], o1[:], s_bc[:, ncols],
                                                op=mybir.AluOpType.mult)
                        nc.sync.dma_start(out[mt * P:(mt + 1) * P, ncols], o2[:])
    nc.compile()
    return nc


def shard_inputs(lhs, rhs, n_cores=8):
    """Full inputs -> per-core in_maps."""
    M = lhs.shape[0] // n_cores
    NSCAN = rhs.shape[1] // n_cores
    return [
        {
            "lhs": np.ascontiguousarray(lhs[c * M:(c + 1) * M]),
            "rhs": rhs,
            "rhs_scan": np.ascontiguousarray(rhs[:, c * NSCAN:(c + 1) * NSCAN]),
        }
        for c in range(n_cores)
    ]


_NC_CACHE = {}


def _get_nc():
    key = "default"
    if key not in _NC_CACHE:
        _NC_CACHE[key] = build(n_cores=N_CORES, M=FULL_M // N_CORES, K=K_DIM, N=N_DIM)
    return _NC_CACHE[key]


def run_sharded(lhs, rhs, trace=False, **kwargs):
    """Run on hardware; returns (full_output, BassKernelResults)."""
    from concourse.bass_utils import run_bass_kernel_spmd
    nc = _get_nc()
    in_maps = shard_inputs(lhs, rhs, N_CORES)
    res = run_bass_kernel_spmd(nc, in_maps, core_ids=list(range(N_CORES)),
                               trace=trace, **kwargs)
    full = np.concatenate([res.results[c]["out"] for c in range(N_CORES)], axis=0)
    return full, res


def kernel(lhs, rhs):
    lhs = np.asarray(lhs, dtype=np.float32)
    rhs = np.asarray(rhs, dtype=np.float32)
    assert lhs.shape == (FULL_M, K_DIM) and rhs.shape == (K_DIM, N_DIM)
    full, _ = run_sharded(lhs, rhs, trace=False)
    return full
